# revision 15
# baseline (speedup 1.0000x reference)
"""Trainium2 Bass kernel for nn_Disentangler (ragged_sequence).

Math (per component-MLP g of 32; 16 node + 16 edge):
    rows  = x[mask]                      # [32768, 2048], row-major
    sel   = rows[idx_g]                  # [1000, 2048]
    h     = gelu(sel @ W1_g + b1_g)      # [1000, 2048]
    y     = h @ W2_g + b2_g              # [1000, 1024]
    pooled= segsum_dedup(y) / TOK        # [16, 1024]
Key folding: the scatter+segment_sum+mean is a linear map P_g ([16,1000],
entries 1/TOK at first-occurrence sample positions, bucketed by timestamp),
so  pooled = (P_g @ h) @ W2_g + (P_g 1) b2_g.  This replaces the second
matmul [1000x2048x1024] (4.2 GF) with a tiny pooling matmul [16x1000x2048]
plus [16x2048x1024] (~0.13 GF).

Distribution: expert parallelism -- 4 components per NeuronCore (8 cores),
no collectives. The host gathers sel^T per component (the only part of x a
core needs), builds P_g (dedup + /TOK baked in), and feeds per-core weights.

Precision: MM1 (the 8.4 GF/component bulk) runs in fp8e4 (e4m3) DoubleRow.
sel is pre-scaled by 2^4 and W1 by 2^9 on the host (power-of-two scales
keep dequant exact; W1 ~ 0.02*N(0,1) would otherwise land in e4m3's
subnormal range). The 2^-13 dequant rides the gelu activation's scale.
h and the pooling matrix are fp8; W2/pht stay bf16. End-to-end rel err
1.50e-2 vs the 2e-2 gate.

On-chip schedule (per component, HW-measured rationale in parens):
    MM1:  h[m-pair, 256nb] = selT^T @ W1, fp8 DoubleRow, K=2048 as 8
          pair-chunks, N=256 moving streams (measured 81 ns/ld+mm pair on
          hw -- near roofline; N=512 streams and ldweights-dedup surgery
          both measured SLOWER, 264/377 ns/mm, so N=256 fresh-stationary
          is kept).
    PSUM: two m-chunks share one [128, 2, 256] bank so ONE gelu evicts a
          pair (halves ScalarE instruction count; ScalarE was ~100us/rep
          busy with half of it per-instruction overhead).
    gelu: ScalarE exact Gelu, scale=2^-13, PSUM -> SBUF fp8, contiguous
          [128, MT, 256] h2 tile per n-block.
    pool: poolPhT[128s, 16] += h2_m^T @ PT_m, plain fp8 (non-DoubleRow:
          full 128-col stationaries get the compiler's Fast Weight Load;
          DR ldweights loads 256 cols at ~2x cost for a 16-col stream).
    miniW2: pooled[16,1024] += poolPhT_chunk^T @ W2_chunk (bf16, N=512),
          PSUM-accumulated across n-blocks, deferred one n-block so the
          DVE pht copy overlaps PE work. (An N=256 split of these plus
          w2-DMA on the Act HWDGE ring regressed to 306us with wrong
          results -- reverted.)
    DMA:  one selT DMA + 2 w1 + 2 w2 halves per component (InstDMACopy on
          the single SP HWDGE ring pays ~625ns fixed each, FIFO; merging
          85 -> ~22 DMAs/rep cut the measured no-compute floor ~170us).
b1/b2 are structurally zero for this problem; b2 != 0 would be corrected on
the host (linear), b1 != 0 is rejected.

Measured (slope method, 8 cores): baseline 338us -> this version 232us.
"""

import numpy as np
import ml_dtypes

import concourse.bacc as bacc
import concourse.mybir as mybir
from concourse.tile import TileContext

F8 = mybir.dt.float8e4
BF = mybir.dt.bfloat16
F32 = mybir.dt.float32
GELU = mybir.ActivationFunctionType.Gelu
DR = mybir.MatmulPerfMode.DoubleRow

NP_F8 = ml_dtypes.float8_e4m3
NP_BF = ml_dtypes.bfloat16

T, TOK, D = 16, 4096, 2048
C = D // 2            # 1024
H = 2 * C             # hidden dim = 2048
K = 16                # components per branch
L = 1000              # samples per component
NN = T * (TOK // 2)   # 32768 rows per branch
NCORES = 8
CPC = (2 * K) // NCORES   # components per core = 4

P = 128
KD = D // P           # 16 contraction chunks over D
NBW = 256             # H-columns per n-block (DoubleRow moving max = 2*256)
NB = H // NBW         # 8 n-blocks
LP = 1024             # L padded: DoubleRow ldweights requires M == 128 exactly
MT = 8                # row chunks
MW = LP // MT         # 128 rows per chunk
SELQ = 4              # selT split into 4 quarters of 4 k-chunks each
KQ = KD // SELQ       # 4

S_SEL = 16.0          # 2^4 host pre-scale on sel before fp8 quantization
S_W1 = 512.0          # 2^9 host pre-scale on W1 before fp8 quantization
DEQUANT = 1.0 / (S_SEL * S_W1)   # 2^-13, applied inside the gelu activation


def build_nc(repeat=1, act=GELU, parts=("mm1", "pool", "w2")):
    """repeat>1 re-emits the whole compute body; used only for timing
    (slope of wall-clock vs repeat cancels fixed dispatch overheads).
    act: override the activation (CoreSim lacks Gelu; sim checks use Tanh).
    parts: which compute sections to emit (timing decomposition only —
    removing a part leaves downstream consumers reading garbage)."""
    nc = bacc.Bacc(None)
    # sel stationary layout: k-pairs contiguous per m-chunk (ISA requires the
    # DoubleRow stationary AP to be [128p, 2, 128] with the pair packed).
    selT = nc.dram_tensor(
        "selT", [CPC, SELQ, P, 2, MT, 2, MW], F8, kind="ExternalInput"
    )
    w1 = nc.dram_tensor("w1", [CPC, NB, P, KD, NBW], F8, kind="ExternalInput")
    w2 = nc.dram_tensor("w2", [CPC, NB, P, 2, C], BF, kind="ExternalInput")
    # pooling matrix, fp8 with entries 1.0 (the 1/TOK is folded into W2 on
    # the host); plain per-row-chunk layout (pool runs non-DR fp8 + FWL)
    pt = nc.dram_tensor("pt", [CPC, MW, MT, 16], F8, kind="ExternalInput")
    out = nc.dram_tensor("out", [CPC, T, C], F32, kind="ExternalOutput")

    with TileContext(nc) as tc:
        with (
            tc.tile_pool(name="selp", bufs=2) as selp,
            tc.tile_pool(name="w1p", bufs=3) as w1p,
            tc.tile_pool(name="w2p", bufs=3) as w2p,
            tc.tile_pool(name="hp", bufs=3) as hp,
            tc.tile_pool(name="smallp", bufs=4) as smallp,
            tc.tile_pool(name="outp", bufs=2) as outp,
            tc.tile_pool(name="cstp", bufs=1) as cstp,
            tc.tile_pool(name="ps_h", bufs=4, space="PSUM") as ps_h,
            tc.tile_pool(name="ps_pool", bufs=2, space="PSUM") as ps_pool,
            tc.tile_pool(name="ps_out", bufs=1, space="PSUM") as ps_out,
        ):
            pt_all = cstp.tile([MW, CPC, MT, 16], F8, tag="pt_all")
            nc.sync.dma_start(pt_all[:], pt.rearrange("c ki m t -> ki c m t"))

            for c_rep in range(repeat * CPC):
                c = c_rep % CPC
                # Few, large DMAs (each InstDMACopy on the single HWDGE ring
                # pays ~625ns fixed): whole selT in 1, w1/w2 in halves.
                sel_sb = selp.tile([P, SELQ, 2, MT, 2, MW], F8, tag="sel")
                nc.sync.dma_start(
                    sel_sb[:], selT[c].rearrange("q p a m o w -> p q a m o w")
                )
                w1_h = []
                for hv in range(2):
                    t = w1p.tile([P, NB // 2, KD, NBW], F8, tag="w1")
                    nc.sync.dma_start(
                        t[:],
                        w1[c, 4 * hv : 4 * hv + 4].rearrange(
                            "nb p k w -> p nb k w"
                        ),
                    )
                    w1_h.append(t)
                w2_h = []
                for hv in range(2):
                    t = w2p.tile([P, NB // 2, 2, C], BF, tag="w2")
                    nc.scalar.dma_start(
                        t[:],
                        w2[c, 4 * hv : 4 * hv + 4].rearrange(
                            "nb p i cc -> p nb i cc"
                        ),
                    )
                    w2_h.append(t)
                pt_sb = pt_all[:, c]
                po_ps = ps_out.tile([16, C], F32, tag="pops")

                pending = None  # deferred (pht_sb, w2tile, nbslot, nb)
                for nb in range(NB):
                    w1_sb = w1_h[nb // 4]
                    w2_sb = w2_h[nb // 4]
                    nbs = nb % 4

                    # --- MM1 + gelu: m-chunk PAIRS share one PSUM bank
                    # ([128, 2, 256]); one gelu per pair -> h2 plain layout ---
                    h2_nb = hp.tile([MW, MT, NBW], F8, tag="h2", name="h2")
                    for mp in range(MT // 2):
                        h_pair = ps_h.tile(
                            [MW, 2, NBW], F32, tag="hps", name="h_pair"
                        )
                        for mo in range(2):
                            m = 2 * mp + mo
                            njmax = KD // 2 if "mm1" in parts else 1
                            for j in range(njmax):
                                q, jp = j // 2, j % 2
                                nc.tensor.matmul(
                                    h_pair[:, mo],
                                    sel_sb[:, q, jp, m],
                                    w1_sb[:, nbs, 2 * j : 2 * j + 2],
                                    start=(j == 0),
                                    stop=(j == njmax - 1),
                                    perf_mode=DR,
                                )
                        nc.scalar.activation(
                            h2_nb[:, 2 * mp : 2 * mp + 2],
                            h_pair[:],
                            act,
                            scale=DEQUANT,
                        )

                    # --- pool: poolPhT[128s, 16] = h2_m^T @ PT_m, fp8 FWL
                    # (non-DR: full 128-col stationaries, tiny moving) ---
                    pool_ps = ps_pool.tile([P, 32], F32, tag="poolps")
                    # DoubleRow over m-chunk pairs: h2 [128, 2, 128] slices are
                    # the DR stationary pack, pt [128, 2, 16] the moving pack
                    # (same pairing structure as MM1's k-chunks).
                    nmp = MT // 2 if "pool" in parts else 1
                    for s in range(2):
                        for mp in range(nmp):
                            nc.tensor.matmul(
                                pool_ps[:, 16 * s : 16 * (s + 1)],
                                h2_nb[:, 2 * mp : 2 * mp + 2, 128 * s : 128 * (s + 1)],
                                pt_sb[:, 2 * mp : 2 * mp + 2],
                                start=(mp == 0),
                                stop=(mp == nmp - 1),
                                perf_mode=DR,
                            )
                    pht_sb = smallp.tile([P, 32], BF, tag="pht")
                    nc.vector.tensor_copy(pht_sb[:], pool_ps[:])

                    # --- deferred miniW2 of previous block (its pht copy has
                    # had this block's MM1+pool to land) ---
                    if pending is not None:
                        pht_prev, w2_prev, nbs_prev, nb_prev = pending
                        ni = 2 if "w2" in parts else 1
                        for hh in range(2):
                            for i in range(ni):
                                nc.tensor.matmul(
                                    po_ps[:, 512 * hh : 512 * (hh + 1)],
                                    pht_prev[:, 16 * i : 16 * (i + 1)],
                                    w2_prev[:, nbs_prev, i, 512 * hh : 512 * (hh + 1)],
                                    start=(nb_prev == 0 and i == 0),
                                    stop=False,
                                )
                    pending = (pht_sb, w2_sb, nbs, nb)

                # --- flush last block's miniW2 ---
                pht_prev, w2_prev, nbs_prev, nb_prev = pending
                ni = 2 if "w2" in parts else 1
                for hh in range(2):
                    for i in range(ni):
                        nc.tensor.matmul(
                            po_ps[:, 512 * hh : 512 * (hh + 1)],
                            pht_prev[:, 16 * i : 16 * (i + 1)],
                            w2_prev[:, nbs_prev, i, 512 * hh : 512 * (hh + 1)],
                            start=False,
                            stop=(i == ni - 1),
                        )

                out_sb = outp.tile([T, C], F32, tag="out")
                nc.vector.tensor_copy(out_sb[:], po_ps[:])
                nc.sync.dma_start(out[c], out_sb[:])

    nc.finalize()
    return nc


_CACHED_NC = None
_RUNNER = None


def _get_nc():
    global _CACHED_NC
    if _CACHED_NC is None:
        _CACHED_NC = build_nc()
    return _CACHED_NC


def _get_runner():
    """Compile once per process: a jitted shard_map over the 8 cores that
    executes the Bass program (mirrors bass2jax.run_bass_via_pjrt's multi-core
    path, without the profiling code paths)."""
    global _RUNNER
    if _RUNNER is not None:
        return _RUNNER
    import jax
    from jax.sharding import Mesh, PartitionSpec
    from jax.experimental.shard_map import shard_map
    from concourse import bass2jax

    nc = _get_nc()
    bass2jax.install_neuronx_cc_hook()
    partition_name = nc.partition_id_tensor.name if nc.partition_id_tensor else None
    in_names, out_names, out_avals, zero_outs = [], [], [], []
    for alloc in nc.m.functions[0].allocations:
        if not isinstance(alloc, mybir.MemoryLocationSet):
            continue
        name = alloc.memorylocations[0].name
        if alloc.kind == "ExternalInput":
            if name != partition_name:
                in_names.append(name)
        elif alloc.kind == "ExternalOutput":
            out_names.append(name)
            shape = tuple(alloc.tensor_shape)
            dtype = mybir.dt.np(alloc.dtype)
            out_avals.append(jax.core.ShapedArray(shape, dtype))
            zero_outs.append(np.zeros(shape, dtype))
    n_params = len(in_names)
    n_outs = len(out_avals)
    all_names = list(in_names) + list(out_names)
    if partition_name is not None:
        all_names.append(partition_name)

    def _body(*args):
        operands = list(args)
        if partition_name is not None:
            operands.append(bass2jax.partition_id_tensor())
        outs = bass2jax._bass_exec_p.bind(
            *operands,
            out_avals=tuple(out_avals),
            in_names=tuple(all_names),
            out_names=tuple(out_names),
            lowering_input_output_aliases=(),
            sim_require_finite=True,
            sim_require_nnan=True,
            nc=nc,
        )
        return tuple(outs)

    devices = jax.devices()[:NCORES]
    assert len(devices) == NCORES, f"need {NCORES} devices, got {len(jax.devices())}"
    mesh = Mesh(np.asarray(devices), ("core",))
    sharded = jax.jit(
        shard_map(
            _body,
            mesh=mesh,
            in_specs=(PartitionSpec("core"),) * (n_params + n_outs),
            out_specs=(PartitionSpec("core"),) * n_outs,
            check_rep=False,
        ),
        donate_argnums=tuple(range(n_params, n_params + n_outs)),
        keep_unused=True,
    )
    _RUNNER = (sharded, in_names, out_names, zero_outs)
    return _RUNNER


def run_spmd(in_maps):
    """Execute on all 8 cores; returns per-core {tensor_name: array}."""
    sharded, in_names, out_names, zero_outs = _get_runner()
    concat_in = [
        np.concatenate([np.asarray(in_maps[c][n]) for c in range(NCORES)], axis=0)
        for n in in_names
    ]
    zeros = [np.concatenate([z] * NCORES, axis=0) for z in zero_outs]
    outs = sharded(*concat_in, *zeros)
    results = []
    for c in range(NCORES):
        per = {}
        for i, n in enumerate(out_names):
            full = np.asarray(outs[i])
            per_core = full.shape[0] // NCORES
            per[n] = full[c * per_core : (c + 1) * per_core]
        results.append(per)
    return results


def prepare_inputs(inputs):
    """Host-side sharding: gather selT, quantize, build pooling matrices.

    Layouts are chosen so every DMA reads fully-contiguous 4KB-per-partition
    lines:  selT [CPC,SELQ,P,KQ,L] fp8, w1 [CPC,NB,P,KD,NBW] fp8,
    w2 [CPC,NB,P,2,C] bf16, pt [CPC,MW,MT,16] bf16.

    Returns (in_maps, b2_corrections) where b2_corrections[g] is the host-side
    rank-1 term cnt_t (x) b2_g / TOK to add for nonzero b2 (zero here).
    """
    x = np.ascontiguousarray(np.asarray(inputs["x"], dtype=np.float32))
    nm = np.asarray(inputs["padded_node_mask"])
    em = np.asarray(inputs["padded_edge_mask"])
    ridx = np.asarray(inputs["rand_indices"])

    node_W1 = np.asarray(inputs["node_W1"], dtype=np.float32)
    node_W2 = np.asarray(inputs["node_W2"], dtype=np.float32)
    edge_W1 = np.asarray(inputs["edge_W1"], dtype=np.float32)
    edge_W2 = np.asarray(inputs["edge_W2"], dtype=np.float32)
    for bname in ("node_b1", "node_b2", "edge_b1", "edge_b2"):
        b = np.asarray(inputs[bname])
        if bname.endswith("b1") and np.any(b):
            raise NotImplementedError("nonzero b1 not supported by this kernel")

    xf = x.reshape(T * TOK, D)
    nt, ntok = np.nonzero(nm)
    et, etok = np.nonzero(em)
    assert nt.size == NN and et.size == NN, "unexpected mask population"
    flat_n = nt * TOK + ntok
    flat_e = et * TOK + etok

    in_maps = []
    b2_corr = np.zeros((2 * K, T, C), np.float32)
    any_b2 = np.any(inputs["node_b2"]) or np.any(inputs["edge_b2"])
    for core in range(NCORES):
        sel_list, pt_list = [], []
        for j in range(CPC):
            g = core * CPC + j
            if g < K:
                flat, seg, b2 = flat_n, nt, np.asarray(inputs["node_b2"])[g]
            else:
                flat, seg, b2 = flat_e, et, np.asarray(inputs["edge_b2"])[g - K]
            idx = ridx[g]
            selpad = np.zeros((LP, D), np.float32)
            selpad[:L] = xf[flat[idx]]
            # [m, col, q, jp, t, p] -> [q, p, jp, m, t, col]
            sel_q = (selpad * S_SEL).astype(NP_F8)
            sel_list.append(
                np.ascontiguousarray(
                    sel_q.reshape(MT, MW, SELQ, 2, 2, P).transpose(2, 5, 3, 0, 4, 1)
                )
            )
            pt_mat = np.zeros((LP, 16), np.float32)
            _, first = np.unique(idx, return_index=True)
            tvals = seg[idx[first]]
            pt_mat[first, tvals] = 1.0   # 1/TOK folded into W2 (exact pow2)
            pt_list.append(
                pt_mat.reshape(MT, MW, 16).transpose(1, 0, 2).astype(NP_F8)
            )
            if any_b2:
                cnt = np.bincount(tvals, minlength=T).astype(np.float32)
                b2_corr[g] = np.outer(cnt / TOK, b2.astype(np.float32))
        if core * CPC < K:
            w1f = node_W1[core * CPC : core * CPC + CPC]
            w2f = node_W2[core * CPC : core * CPC + CPC]
        else:
            o = core * CPC - K
            w1f = edge_W1[o : o + CPC]
            w2f = edge_W2[o : o + CPC]
        # w1: [CPC, D, H] -> [CPC, NB, P, KD, NBW], quantized fp8 at 2^9
        w1v = np.ascontiguousarray(
            (w1f * S_W1)
            .astype(NP_F8)
            .reshape(CPC, KD, P, NB, NBW)
            .transpose(0, 3, 2, 1, 4)
        )
        # w2: [CPC, H, C] -> [CPC, NB, P, 2, C] bf16, carrying the pool's 1/TOK
        w2v = np.ascontiguousarray(
            (w2f / TOK).astype(NP_BF).reshape(CPC, NB, 2, P, C).transpose(0, 1, 3, 2, 4)
        )
        in_maps.append(
            {
                "selT": np.ascontiguousarray(np.stack(sel_list)),
                "w1": w1v,
                "w2": w2v,
                "pt": np.ascontiguousarray(np.stack(pt_list)),
            }
        )
    return in_maps, b2_corr


def assemble_output(results, b2_corr):
    comp_all = np.empty((2 * K, T, C), np.float32)
    for core in range(NCORES):
        comp_all[core * CPC : (core + 1) * CPC] = results[core]["out"]
    comp_all += b2_corr
    return np.ascontiguousarray(comp_all.transpose(1, 0, 2).reshape(T, 1, 2 * K * C))


def kernel(**inputs) -> np.ndarray:
    in_maps, b2_corr = prepare_inputs(inputs)
    results = run_spmd(in_maps)
    return assemble_output(results, b2_corr)



# revision 16
# speedup vs baseline: 1.2257x; 1.2257x over previous
"""Trainium2 Bass kernel for nn_Disentangler (ragged_sequence).

Math (per component-MLP g of 32; 16 node + 16 edge):
    rows  = x[mask]                      # [32768, 2048], row-major
    sel   = rows[idx_g]                  # [1000, 2048]
    h     = gelu(sel @ W1_g + b1_g)      # [1000, 2048]
    y     = h @ W2_g + b2_g              # [1000, 1024]
    pooled= segsum_dedup(y) / TOK        # [16, 1024]
Key folding: the scatter+segment_sum+mean is a linear map P_g ([16,1000],
entries 1/TOK at first-occurrence sample positions, bucketed by timestamp),
so  pooled = (P_g @ h) @ W2_g + (P_g 1) b2_g.  This replaces the second
matmul [1000x2048x1024] (4.2 GF) with a tiny pooling matmul [16x1000x2048]
plus [16x2048x1024] (~0.13 GF).

Distribution: expert parallelism -- 4 components per NeuronCore (8 cores),
no collectives. The host gathers sel^T per component (the only part of x a
core needs), builds P_g (dedup + /TOK baked in), and feeds per-core weights.

Precision: MM1 (the 8.4 GF/component bulk) runs in fp8e4 (e4m3) DoubleRow.
sel is pre-scaled by 2^4 and W1 by 2^9 on the host (power-of-two scales
keep dequant exact; W1 ~ 0.02*N(0,1) would otherwise land in e4m3's
subnormal range). The 2^-13 dequant rides the gelu activation's scale.
h and the pooling matrix are fp8; W2/pht stay bf16. End-to-end rel err
1.50e-2 vs the 2e-2 gate.

On-chip schedule (per component, HW-measured rationale in parens):
    MM1:  h[m-pair, 256nb] = selT^T @ W1, fp8 DoubleRow, K=2048 as 8
          pair-chunks, N=256 moving streams (measured 81 ns/ld+mm pair on
          hw -- near roofline; N=512 streams and ldweights-dedup surgery
          both measured SLOWER, 264/377 ns/mm, so N=256 fresh-stationary
          is kept).
    PSUM: two m-chunks share one [128, 2, 256] bank so ONE gelu evicts a
          pair (halves ScalarE instruction count; ScalarE was ~100us/rep
          busy with half of it per-instruction overhead).
    gelu: ScalarE exact Gelu, scale=2^-13, PSUM -> SBUF fp8, contiguous
          [128, MT, 256] h2 tile per n-block.
    pool: poolPhT[128s, 16] += h2_m^T @ PT_m, plain fp8 (non-DoubleRow:
          full 128-col stationaries get the compiler's Fast Weight Load;
          DR ldweights loads 256 cols at ~2x cost for a 16-col stream).
    miniW2: pooled[16,1024] += poolPhT_chunk^T @ W2_chunk (bf16, N=512),
          PSUM-accumulated across n-blocks, deferred one n-block so the
          DVE pht copy overlaps PE work. (An N=256 split of these plus
          w2-DMA on the Act HWDGE ring regressed to 306us with wrong
          results -- reverted.)
    DMA:  one selT DMA + 2 w1 + 2 w2 halves per component (InstDMACopy on
          the single SP HWDGE ring pays ~625ns fixed each, FIFO; merging
          85 -> ~22 DMAs/rep cut the measured no-compute floor ~170us).
b1/b2 are structurally zero for this problem; b2 != 0 would be corrected on
the host (linear), b1 != 0 is rejected.

Measured (slope method, 8 cores): baseline 338us -> this version 232us.
"""

import numpy as np
import ml_dtypes

import concourse.bacc as bacc
import concourse.mybir as mybir
from concourse.tile import TileContext

F8 = mybir.dt.float8e4
BF = mybir.dt.bfloat16
F32 = mybir.dt.float32
GELU = mybir.ActivationFunctionType.Gelu
DR = mybir.MatmulPerfMode.DoubleRow

NP_F8 = ml_dtypes.float8_e4m3
NP_BF = ml_dtypes.bfloat16

T, TOK, D = 16, 4096, 2048
C = D // 2            # 1024
H = 2 * C             # hidden dim = 2048
K = 16                # components per branch
L = 1000              # samples per component
NN = T * (TOK // 2)   # 32768 rows per branch
NCORES = 8
CPC = (2 * K) // NCORES   # components per core = 4

P = 128
KD = D // P           # 16 contraction chunks over D
NBW = 256             # H-columns per n-block (DoubleRow moving max = 2*256)
NB = H // NBW         # 8 n-blocks
LP = 1024             # L padded: DoubleRow ldweights requires M == 128 exactly
MT = 8                # row chunks
MW = LP // MT         # 128 rows per chunk
SELQ = 4              # selT split into 4 quarters of 4 k-chunks each
KQ = KD // SELQ       # 4

S_SEL = 16.0          # 2^4 host pre-scale on sel before fp8 quantization
S_W1 = 512.0          # 2^9 host pre-scale on W1 before fp8 quantization
DEQUANT = 1.0 / (S_SEL * S_W1)   # 2^-13, applied inside the gelu activation


def build_nc(repeat=1, act=GELU, parts=("mm1", "pool", "w2")):
    """repeat>1 re-emits the whole compute body; used only for timing
    (slope of wall-clock vs repeat cancels fixed dispatch overheads).
    act: override the activation (CoreSim lacks Gelu; sim checks use Tanh).
    parts: which compute sections to emit (timing decomposition only —
    removing a part leaves downstream consumers reading garbage)."""
    nc = bacc.Bacc(None)
    # sel stationary layout: k-pairs contiguous per m-chunk (ISA requires the
    # DoubleRow stationary AP to be [128p, 2, 128] with the pair packed).
    selT = nc.dram_tensor(
        "selT", [CPC, SELQ, P, 2, MT, 2, MW], F8, kind="ExternalInput"
    )
    w1 = nc.dram_tensor("w1", [CPC, NB, P, KD, NBW], F8, kind="ExternalInput")
    w2 = nc.dram_tensor("w2", [CPC, NB, P, 2, C], BF, kind="ExternalInput")
    # pooling matrix, fp8 with entries 1.0 (the 1/TOK is folded into W2 on
    # the host); plain per-row-chunk layout (pool runs non-DR fp8 + FWL)
    pt = nc.dram_tensor("pt", [CPC, MW, MT, 16], F8, kind="ExternalInput")
    out = nc.dram_tensor("out", [CPC, T, C], F32, kind="ExternalOutput")

    with TileContext(nc) as tc:
        with (
            tc.tile_pool(name="selp", bufs=2) as selp,
            tc.tile_pool(name="w1p", bufs=3) as w1p,
            tc.tile_pool(name="w2p", bufs=3) as w2p,
            tc.tile_pool(name="hp", bufs=3) as hp,
            tc.tile_pool(name="smallp", bufs=4) as smallp,
            tc.tile_pool(name="outp", bufs=2) as outp,
            tc.tile_pool(name="cstp", bufs=1) as cstp,
            tc.tile_pool(name="ps_h", bufs=4, space="PSUM") as ps_h,
            tc.tile_pool(name="ps_pool", bufs=2, space="PSUM") as ps_pool,
            tc.tile_pool(name="ps_out", bufs=1, space="PSUM") as ps_out,
        ):
            pt_all = cstp.tile([MW, CPC, MT, 16], F8, tag="pt_all")
            nc.sync.dma_start(pt_all[:], pt.rearrange("c ki m t -> ki c m t"))

            for c_rep in range(repeat * CPC):
                c = c_rep % CPC
                # Few, large DMAs (each InstDMACopy on the single HWDGE ring
                # pays ~625ns fixed): whole selT in 1, w1/w2 in halves.
                sel_sb = selp.tile([P, SELQ, 2, MT, 2, MW], F8, tag="sel")
                nc.sync.dma_start(
                    sel_sb[:], selT[c].rearrange("q p a m o w -> p q a m o w")
                )
                w1_h = []
                for hv in range(2):
                    t = w1p.tile([P, NB // 2, KD, NBW], F8, tag="w1")
                    nc.sync.dma_start(
                        t[:],
                        w1[c, 4 * hv : 4 * hv + 4].rearrange(
                            "nb p k w -> p nb k w"
                        ),
                    )
                    w1_h.append(t)
                w2_h = []
                for hv in range(2):
                    t = w2p.tile([P, NB // 2, 2, C], BF, tag="w2")
                    nc.sync.dma_start(
                        t[:],
                        w2[c, 4 * hv : 4 * hv + 4].rearrange(
                            "nb p i cc -> p nb i cc"
                        ),
                    )
                    w2_h.append(t)
                pt_sb = pt_all[:, c]
                po_ps = ps_out.tile([16, C], F32, tag="pops")

                pending = None  # deferred (pht_sb, w2tile, nbslot, nb)
                for nb in range(NB):
                    w1_sb = w1_h[nb // 4]
                    w2_sb = w2_h[nb // 4]
                    nbs = nb % 4

                    # --- MM1 + gelu: m-chunk PAIRS share one PSUM bank
                    # ([128, 2, 256]); one gelu per pair -> h2 plain layout ---
                    h2_nb = hp.tile([MW, MT, NBW], F8, tag="h2", name="h2")
                    for mp in range(MT // 2):
                        h_pair = ps_h.tile(
                            [MW, 2, NBW], F32, tag="hps", name="h_pair"
                        )
                        for mo in range(2):
                            m = 2 * mp + mo
                            njmax = KD // 2 if "mm1" in parts else 1
                            for j in range(njmax):
                                q, jp = j // 2, j % 2
                                nc.tensor.matmul(
                                    h_pair[:, mo],
                                    sel_sb[:, q, jp, m],
                                    w1_sb[:, nbs, 2 * j : 2 * j + 2],
                                    start=(j == 0),
                                    stop=(j == njmax - 1),
                                    perf_mode=DR,
                                )
                        nc.scalar.activation(
                            h2_nb[:, 2 * mp : 2 * mp + 2],
                            h_pair[:],
                            act,
                            scale=DEQUANT,
                        )

                    # --- pool: poolPhT[128s, 16] = h2_m^T @ PT_m, fp8 FWL
                    # (non-DR: full 128-col stationaries, tiny moving) ---
                    pool_ps = ps_pool.tile([P, 32], F32, tag="poolps")
                    nmm = MT if "pool" in parts else 1
                    for s in range(2):
                        for m in range(nmm):
                            nc.tensor.matmul(
                                pool_ps[:, 16 * s : 16 * (s + 1)],
                                h2_nb[:, m, 128 * s : 128 * (s + 1)],
                                pt_sb[:, m],
                                start=(m == 0),
                                stop=(m == nmm - 1),
                            )
                    pht_sb = smallp.tile([P, 32], BF, tag="pht")
                    nc.vector.tensor_copy(pht_sb[:], pool_ps[:])

                    # --- deferred miniW2 of previous block (its pht copy has
                    # had this block's MM1+pool to land) ---
                    if pending is not None:
                        pht_prev, w2_prev, nbs_prev, nb_prev = pending
                        ni = 2 if "w2" in parts else 1
                        for hh in range(2):
                            for i in range(ni):
                                nc.tensor.matmul(
                                    po_ps[:, 512 * hh : 512 * (hh + 1)],
                                    pht_prev[:, 16 * i : 16 * (i + 1)],
                                    w2_prev[:, nbs_prev, i, 512 * hh : 512 * (hh + 1)],
                                    start=(nb_prev == 0 and i == 0),
                                    stop=False,
                                )
                    pending = (pht_sb, w2_sb, nbs, nb)

                # --- flush last block's miniW2 ---
                pht_prev, w2_prev, nbs_prev, nb_prev = pending
                ni = 2 if "w2" in parts else 1
                for hh in range(2):
                    for i in range(ni):
                        nc.tensor.matmul(
                            po_ps[:, 512 * hh : 512 * (hh + 1)],
                            pht_prev[:, 16 * i : 16 * (i + 1)],
                            w2_prev[:, nbs_prev, i, 512 * hh : 512 * (hh + 1)],
                            start=False,
                            stop=(i == ni - 1),
                        )

                out_sb = outp.tile([T, C], F32, tag="out")
                nc.vector.tensor_copy(out_sb[:], po_ps[:])
                nc.sync.dma_start(out[c], out_sb[:])

    nc.finalize()
    return nc


_CACHED_NC = None
_RUNNER = None


def _get_nc():
    global _CACHED_NC
    if _CACHED_NC is None:
        _CACHED_NC = build_nc()
    return _CACHED_NC


def _get_runner():
    """Compile once per process: a jitted shard_map over the 8 cores that
    executes the Bass program (mirrors bass2jax.run_bass_via_pjrt's multi-core
    path, without the profiling code paths)."""
    global _RUNNER
    if _RUNNER is not None:
        return _RUNNER
    import jax
    from jax.sharding import Mesh, PartitionSpec
    from jax.experimental.shard_map import shard_map
    from concourse import bass2jax

    nc = _get_nc()
    bass2jax.install_neuronx_cc_hook()
    partition_name = nc.partition_id_tensor.name if nc.partition_id_tensor else None
    in_names, out_names, out_avals, zero_outs = [], [], [], []
    for alloc in nc.m.functions[0].allocations:
        if not isinstance(alloc, mybir.MemoryLocationSet):
            continue
        name = alloc.memorylocations[0].name
        if alloc.kind == "ExternalInput":
            if name != partition_name:
                in_names.append(name)
        elif alloc.kind == "ExternalOutput":
            out_names.append(name)
            shape = tuple(alloc.tensor_shape)
            dtype = mybir.dt.np(alloc.dtype)
            out_avals.append(jax.core.ShapedArray(shape, dtype))
            zero_outs.append(np.zeros(shape, dtype))
    n_params = len(in_names)
    n_outs = len(out_avals)
    all_names = list(in_names) + list(out_names)
    if partition_name is not None:
        all_names.append(partition_name)

    def _body(*args):
        operands = list(args)
        if partition_name is not None:
            operands.append(bass2jax.partition_id_tensor())
        outs = bass2jax._bass_exec_p.bind(
            *operands,
            out_avals=tuple(out_avals),
            in_names=tuple(all_names),
            out_names=tuple(out_names),
            lowering_input_output_aliases=(),
            sim_require_finite=True,
            sim_require_nnan=True,
            nc=nc,
        )
        return tuple(outs)

    devices = jax.devices()[:NCORES]
    assert len(devices) == NCORES, f"need {NCORES} devices, got {len(jax.devices())}"
    mesh = Mesh(np.asarray(devices), ("core",))
    sharded = jax.jit(
        shard_map(
            _body,
            mesh=mesh,
            in_specs=(PartitionSpec("core"),) * (n_params + n_outs),
            out_specs=(PartitionSpec("core"),) * n_outs,
            check_rep=False,
        ),
        donate_argnums=tuple(range(n_params, n_params + n_outs)),
        keep_unused=True,
    )
    _RUNNER = (sharded, in_names, out_names, zero_outs)
    return _RUNNER


def run_spmd(in_maps):
    """Execute on all 8 cores; returns per-core {tensor_name: array}."""
    sharded, in_names, out_names, zero_outs = _get_runner()
    concat_in = [
        np.concatenate([np.asarray(in_maps[c][n]) for c in range(NCORES)], axis=0)
        for n in in_names
    ]
    zeros = [np.concatenate([z] * NCORES, axis=0) for z in zero_outs]
    outs = sharded(*concat_in, *zeros)
    results = []
    for c in range(NCORES):
        per = {}
        for i, n in enumerate(out_names):
            full = np.asarray(outs[i])
            per_core = full.shape[0] // NCORES
            per[n] = full[c * per_core : (c + 1) * per_core]
        results.append(per)
    return results


def prepare_inputs(inputs):
    """Host-side sharding: gather selT, quantize, build pooling matrices.

    Layouts are chosen so every DMA reads fully-contiguous 4KB-per-partition
    lines:  selT [CPC,SELQ,P,KQ,L] fp8, w1 [CPC,NB,P,KD,NBW] fp8,
    w2 [CPC,NB,P,2,C] bf16, pt [CPC,MW,MT,16] bf16.

    Returns (in_maps, b2_corrections) where b2_corrections[g] is the host-side
    rank-1 term cnt_t (x) b2_g / TOK to add for nonzero b2 (zero here).
    """
    x = np.ascontiguousarray(np.asarray(inputs["x"], dtype=np.float32))
    nm = np.asarray(inputs["padded_node_mask"])
    em = np.asarray(inputs["padded_edge_mask"])
    ridx = np.asarray(inputs["rand_indices"])

    node_W1 = np.asarray(inputs["node_W1"], dtype=np.float32)
    node_W2 = np.asarray(inputs["node_W2"], dtype=np.float32)
    edge_W1 = np.asarray(inputs["edge_W1"], dtype=np.float32)
    edge_W2 = np.asarray(inputs["edge_W2"], dtype=np.float32)
    for bname in ("node_b1", "node_b2", "edge_b1", "edge_b2"):
        b = np.asarray(inputs[bname])
        if bname.endswith("b1") and np.any(b):
            raise NotImplementedError("nonzero b1 not supported by this kernel")

    xf = x.reshape(T * TOK, D)
    nt, ntok = np.nonzero(nm)
    et, etok = np.nonzero(em)
    assert nt.size == NN and et.size == NN, "unexpected mask population"
    flat_n = nt * TOK + ntok
    flat_e = et * TOK + etok

    in_maps = []
    b2_corr = np.zeros((2 * K, T, C), np.float32)
    any_b2 = np.any(inputs["node_b2"]) or np.any(inputs["edge_b2"])
    for core in range(NCORES):
        sel_list, pt_list = [], []
        for j in range(CPC):
            g = core * CPC + j
            if g < K:
                flat, seg, b2 = flat_n, nt, np.asarray(inputs["node_b2"])[g]
            else:
                flat, seg, b2 = flat_e, et, np.asarray(inputs["edge_b2"])[g - K]
            idx = ridx[g]
            selpad = np.zeros((LP, D), np.float32)
            selpad[:L] = xf[flat[idx]]
            # [m, col, q, jp, t, p] -> [q, p, jp, m, t, col]
            sel_q = (selpad * S_SEL).astype(NP_F8)
            sel_list.append(
                np.ascontiguousarray(
                    sel_q.reshape(MT, MW, SELQ, 2, 2, P).transpose(2, 5, 3, 0, 4, 1)
                )
            )
            pt_mat = np.zeros((LP, 16), np.float32)
            _, first = np.unique(idx, return_index=True)
            tvals = seg[idx[first]]
            pt_mat[first, tvals] = 1.0   # 1/TOK folded into W2 (exact pow2)
            pt_list.append(
                pt_mat.reshape(MT, MW, 16).transpose(1, 0, 2).astype(NP_F8)
            )
            if any_b2:
                cnt = np.bincount(tvals, minlength=T).astype(np.float32)
                b2_corr[g] = np.outer(cnt / TOK, b2.astype(np.float32))
        if core * CPC < K:
            w1f = node_W1[core * CPC : core * CPC + CPC]
            w2f = node_W2[core * CPC : core * CPC + CPC]
        else:
            o = core * CPC - K
            w1f = edge_W1[o : o + CPC]
            w2f = edge_W2[o : o + CPC]
        # w1: [CPC, D, H] -> [CPC, NB, P, KD, NBW], quantized fp8 at 2^9
        w1v = np.ascontiguousarray(
            (w1f * S_W1)
            .astype(NP_F8)
            .reshape(CPC, KD, P, NB, NBW)
            .transpose(0, 3, 2, 1, 4)
        )
        # w2: [CPC, H, C] -> [CPC, NB, P, 2, C] bf16, carrying the pool's 1/TOK
        w2v = np.ascontiguousarray(
            (w2f / TOK).astype(NP_BF).reshape(CPC, NB, 2, P, C).transpose(0, 1, 3, 2, 4)
        )
        in_maps.append(
            {
                "selT": np.ascontiguousarray(np.stack(sel_list)),
                "w1": w1v,
                "w2": w2v,
                "pt": np.ascontiguousarray(np.stack(pt_list)),
            }
        )
    return in_maps, b2_corr


def assemble_output(results, b2_corr):
    comp_all = np.empty((2 * K, T, C), np.float32)
    for core in range(NCORES):
        comp_all[core * CPC : (core + 1) * CPC] = results[core]["out"]
    comp_all += b2_corr
    return np.ascontiguousarray(comp_all.transpose(1, 0, 2).reshape(T, 1, 2 * K * C))


def kernel(**inputs) -> np.ndarray:
    in_maps, b2_corr = prepare_inputs(inputs)
    results = run_spmd(in_maps)
    return assemble_output(results, b2_corr)

